# revision 27
# baseline (speedup 1.0000x reference)
"""Self-attention (Q=K=V) Trainium2 Bass kernel.

Full input: inputs [8, 2048, 256] fp32.  Output: softmax(X X^T / 16) X,
batched over dim 0.  Sharding: pure data-parallel - one batch element
per NeuronCore (8 cores), no collectives.

Per-core algorithm (X = [2048, 256]):
  - Load X into SBUF row-block tiles (plus two ones columns); X^T built
    on-chip with fp8 PE transposes (identity streams at 1 col/cycle).
  - Scores run in fp8e4 DoubleRow (one matmul contracts K=256); per
    column group g and row PAIR (2j, 2j+1) the two 512-wide matmuls land
    in adjacent PSUM banks and ONE 1024-wide exp drains them (the ~180
    cycle ACT instruction overhead is paid per 1024 elements, not 512).
    exp is biased by -ln(1024) so off-diagonal E/1024 fits fp8e4; the
    diagonal-group rows go to bf16 (e32) instead.
  - S is symmetric, so E's row-blocks double as the TRANSPOSED
    probability blocks stage 2 needs as stationary operands.
  - Stage 2 (per 128-query tile): 6 fp8 DoubleRow pair-matmuls
    (off-diagonal row pairs, ~4e-4 of softmax mass) + 4 bf16 matmuls
    (diagonal-group rows, ~99.96% of the mass).  The ones columns
    accumulate the denominator in the same PSUM tile, bit-consistent
    with the numerator weights.  Scale by its reciprocal and DMA out.
  - Emission is readiness-driven: each phase's stage-2 work starts as
    soon as the needed exp rows exist (diag row-pairs go first in each
    phase), so the post-loop tail is only a few matmuls.  Dummy ident
    matmuls warm the PE clock gate and a dummy exp preloads the ACT
    table set during the input-DMA wait.
"""

import numpy as np

import concourse.bacc as bacc
import concourse.tile as tile
from concourse import mybir
from concourse.bass_utils import run_bass_kernel_spmd
from concourse.masks import make_identity

B = 8
N = 2048
D = 256
P = 128
T = N // P   # 16 row/column tiles
C = D // P   # 2 contraction chunks for the scores matmul
G = 4        # 512-wide column groups
GW = N // G  # 512
NP8 = 8      # row pairs
SCALE = 1.0 / 16.0  # 1/sqrt(D)
EBIAS = -6.931471805599453  # -ln(1024): off-diag E fits fp8e4 up to s~12.4

F32 = mybir.dt.float32
BF16 = mybir.dt.bfloat16
FP8 = mybir.dt.float8e4
DR = mybir.MatmulPerfMode.DoubleRow
EXPF = mybir.ActivationFunctionType.Exp


def _row_pairs(g):
    """Phase-g row-pair order: the two diagonal-group pairs first, so
    e32 rows for this group's output tiles are exp'd early."""
    dp = [2 * g, 2 * g + 1]
    return dp + [p for p in range(NP8) if p not in dp]


def _build_nc():
    nc = bacc.Bacc("TRN2", target_bir_lowering=False, debug=False, num_devices=B)
    x = nc.dram_tensor("x", [N, D], F32, kind="ExternalInput").ap()
    out = nc.dram_tensor("out", [N, D], F32, kind="ExternalOutput").ap()

    with tile.TileContext(nc) as tc:
        with (
            tc.tile_pool(name="big", bufs=1) as big,
            tc.tile_pool(name="small", bufs=1) as small,
            tc.tile_pool(name="psum", bufs=2, space="PSUM") as psum,
            tc.tile_pool(name="ot", bufs=8) as ot,
        ):
            # x_tiles[j][p, 0:256] = X[j*128+p, :]; cols 256:258 = 1.0
            x_tiles = [
                big.tile([P, D + 2], F32, name=f"xj{j}", tag=f"x{j}")
                for j in range(T)
            ]
            xr_tiles = [
                big.tile([P, D + 2], BF16, name=f"xr{j}", tag=f"xr{j}")
                for j in range(T)
            ]
            # X^T: xt[p, c, n] = X[n, c*128+p]
            xt_sb = big.tile([P, C, N], FP8, tag="xt")
            # E/1024 storage: the diagonal-group tile of each row is bf16
            # (carries ~99.96% of softmax mass); off-diagonal tiles fp8.
            e32 = big.tile([P, T, GW], BF16, tag="e32")
            e8 = big.tile([P, T, N], FP8, tag="e8")
            # fp8 x pairs for DoubleRow stage-2: x8p[v][p,h,:] = row 2v+h
            x8p = [
                big.tile([P, 2, D + 2], FP8, name=f"x8p{v}", tag=f"x8p{v}")
                for v in range(NP8)
            ]

            ident32 = small.tile([P, P], F32, tag="id32")
            ident8 = small.tile([P, P], FP8, tag="id8")
            ebias = small.tile([P, 1], F32, tag="eb")
            wsrc = small.tile([P, P], FP8, tag="wsrc")

            # gpsimd's Q7 cores take ~6us to boot, so only the identity
            # (first needed by the ~6.3us transposes) may depend on it;
            # warmup-matmul and exp constants come from the DVE.
            nc.vector.memset(wsrc[:], 1.0)
            nc.vector.memset(ebias[:], EBIAS)
            make_identity(nc, ident32)
            nc.vector.tensor_copy(ident8[:], ident32[:])

            # Input DMAs.  Only sync/scalar/gpsimd queues can issue DMAs
            # and one dma_start transfers on ONE of the 16 DMA engines
            # (~22.5 GB/s, so a full 131KB tile takes ~5.8us).  Early
            # tiles are split into partition halves on two queues; the
            # scalar queue must stay clear for the exp stream, so its few
            # issues are interleaved between the first exps (below).
            xv = x.rearrange("(t p) d -> p t d", p=P)

            def in_dma(eng, j, p0, p1):
                eng.dma_start(out=x_tiles[j][p0:p1, 0:D], in_=xv[p0:p1, j, :])

            H = 64
            # both halves of each tile issued back-to-back so two DMA
            # engines transfer a tile in parallel (~2.75us instead of ~5.8)
            for j in [0, 2, 4, 6, 8, 10, 12, 14]:
                in_dma(nc.sync, j, 0, H)
                in_dma(nc.sync, j, H, P)
            for j in [1, 3, 5]:
                in_dma(nc.scalar, j, 0, H)
                in_dma(nc.scalar, j, H, P)
            # gpsimd boots ~6us in; its issues are slower (SWDGE ~1us)
            for j in [7, 9, 15, 11, 13]:
                in_dma(nc.gpsimd, j, 0, H)
                in_dma(nc.gpsimd, j, H, P)

            for j in range(T):
                nc.vector.memset(x_tiles[j][:, D : D + 2], 1.0)

            # PE clock-gate warmup: dummy matmuls keep the PE busy through
            # the HAM activity window during the DMA wait, so the real
            # matmul stream starts at 2.4 GHz.
            warm_n = [0]

            def warm(n):
                # dummy matmuls keep the PE inside the HAM activity window
                # (idle >3.4us re-throttles the clock to 1.2 GHz)
                for _ in range(n):
                    w = warm_n[0]
                    warm_n[0] += 1
                    wp = psum.tile([P, P], F32, tag="sm", bufs=4,
                                   name=f"wp{w}", padded_shape=(P, 512))
                    nc.tensor.matmul(
                        wp[:], lhsT=wsrc[:], rhs=wsrc[:], start=True,
                        stop=True,
                    )

            warm(40)

            xr_emitted = set()

            def emit_xr(j):
                # xr casts are lazy (phase-ahead of their stage-2 use) to
                # keep them out of the phase-0 DVE stream.  They must NOT
                # go on gpsimd: concurrent gpsimd+DVE reads of the same
                # x_tiles quarter the DVE cast rate (SBUF port contention).
                if j not in xr_emitted:
                    xr_emitted.add(j)
                    nc.vector.tensor_copy(xr_tiles[j][:], x_tiles[j][:])

            def load_step(j):
                # x8p (fp8) feeds both the stage-2 rhs and the fp8 PE
                # transposes that build X^T; xr (bf16) feeds stage-2 diag.
                nc.vector.tensor_copy(x8p[j // 2][:, j % 2, :], x_tiles[j][:])
                for c in range(C):
                    # fp8 transpose-mode requires an output element step of
                    # 2 in PSUM, hence the [P, P, 2] scratch written at
                    # stride 2.
                    pt = psum.tile([P, P, 2], FP8, tag="sm", bufs=4,
                                   name=f"pt{j}_{c}", padded_shape=(P, 1024, 2))
                    nc.tensor.transpose(
                        pt[:, :, 0], x8p[j // 2][:, j % 2, c * P : (c + 1) * P],
                        ident8[:],
                    )
                    nc.vector.tensor_copy(
                        xt_sb[:, c, j * P : (j + 1) * P], pt[:, :, 0]
                    )

            def t1_pair(g, p):
                """Scores + exp for row pair (2p, 2p+1), column group g.
                Two DoubleRow matmuls into adjacent PSUM banks, one
                1024-wide exp over both."""
                r0 = 2 * p
                ps = psum.tile([P, 2, GW], F32, tag="pair", name=f"ps{g}_{p}")
                for h in range(2):
                    nc.tensor.matmul(
                        ps[:, h, :],
                        lhsT=xt_sb[:, :, (r0 + h) * P : (r0 + h + 1) * P],
                        rhs=xt_sb[:, :, g * GW : (g + 1) * GW],
                        start=True,
                        stop=True,
                        perf_mode=DR,
                    )
                dst = (
                    e32[:, r0 : r0 + 2, :]
                    if p in (2 * g, 2 * g + 1)
                    else e8[:, r0 : r0 + 2, g * GW : (g + 1) * GW]
                )
                nc.scalar.activation(
                    out=dst, in_=ps[:, :, :], func=EXPF,
                    scale=SCALE, bias=ebias[:],
                )

            # ---- stage 2 bookkeeping -------------------------------------
            out_r = out.rearrange("(t p) d -> p t d", p=P)
            exp_done = [set() for _ in range(G)]  # pairs exp'd per phase
            NS2 = 10

            class S2:
                __slots__ = ("it", "g", "steps", "emitted", "po")

            tiles = []
            for it in range(T):
                g = it // 4
                t = S2()
                t.it, t.g, t.emitted, t.po = it, g, 0, None
                t.steps = [("bf", j) for j in range(4 * g, 4 * g + 4)] + [
                    ("f8", p) for p in _row_pairs(g) if p not in (2 * g, 2 * g + 1)
                ]
                tiles.append(t)

            out_engs = [nc.sync, nc.gpsimd]

            def close(t):
                it = t.it
                rl = ot.tile([P, 1], F32, tag="rl", name=f"rl{it}")
                nc.vector.reciprocal(rl[:], t.po[:, D : D + 1])
                o_t = ot.tile([P, D], F32, tag="ot", name=f"o{it}")
                if it < T - 4:
                    nc.vector.tensor_scalar_mul(o_t[:], t.po[:, 0:D], rl[:])
                    eng = out_engs[it % 2]
                    eng.dma_start(out=out_r[:, it, :], in_=o_t[:])
                else:
                    # the last 4 tiles close after the final exp: normalize
                    # on the now-idle scalar engine so the 4 closes don't
                    # serialize on the DVE, and spread the half-tile DMA
                    # issues over three queues so scalar's COPY stream and
                    # the issue serialization don't add up
                    nc.scalar.mul(o_t[:], t.po[:, 0:D], rl[:])
                    beng = nc.gpsimd if it < T - 2 else nc.scalar
                    for (q0, q1), eng in [((0, H), nc.sync), ((H, P), beng)]:
                        eng.dma_start(
                            out=out_r[q0:q1, it, :], in_=o_t[q0:q1, :]
                        )

            def step_ready(t, s):
                kind, v = t.steps[s]
                pair = v // 2 if kind == "bf" else v
                return pair in exp_done[t.g]

            def emit_one(t):
                kind, v = t.steps[t.emitted]
                first = t.emitted == 0
                last = t.emitted == NS2 - 1
                if first:
                    t.po = psum.tile([P, D + 2], F32, tag="sm", bufs=4,
                                     name=f"po{t.it}", padded_shape=(P, 512))
                if kind == "f8":
                    nc.tensor.matmul(
                        t.po[:],
                        lhsT=e8[:, 2 * v : 2 * v + 2,
                                t.it * P : (t.it + 1) * P],
                        rhs=x8p[v][:],
                        start=first, stop=last, perf_mode=DR,
                    )
                else:
                    lo = (t.it % 4) * P
                    nc.tensor.matmul(
                        t.po[:],
                        lhsT=e32[:, v, lo : lo + P],
                        rhs=xr_tiles[v][:],
                        start=first, stop=last,
                    )
                t.emitted += 1
                if t.emitted == NS2:
                    close(t)

            def open_count():
                return sum(1 for t in tiles if 0 < t.emitted < NS2)

            def pump(budget, max_open=4):
                emitted = 0
                while budget is None or emitted < budget:
                    progressed = False
                    for t in tiles:
                        if t.emitted >= NS2:
                            continue
                        if t.emitted == 0 and open_count() >= max_open:
                            break
                        if step_ready(t, t.emitted):
                            emit_one(t)
                            emitted += 1
                            progressed = True
                            break
                    if not progressed:
                        break
                return emitted

            # ---- schedule ------------------------------------------------
            for j in range(4):
                load_step(j)
            # phase 0: loads + scores; once the loads are emitted (their
            # PSUM scratch tag frees up) group-0 stage-2 starts eagerly in
            # the input-limited PE idle time.  Warmup matmuls fill the
            # rest so the PE clock gate never re-throttles.
            for k, p in enumerate(_row_pairs(0)):
                for j in (2 * k + 4, 2 * k + 5):
                    if 4 <= j < T:
                        load_step(j)
                t1_pair(0, p)
                exp_done[0].add(p)
                if k == 0:
                    for j in range(4):
                        emit_xr(j)
                if k < 6:
                    warm(3)
                else:
                    pump(5, max_open=2)
            pump(10, max_open=3)  # group-0 bonus before phase 1 starts
            # phases 1-3: interleave ready stage-2 work after each pair
            total_mm = T * NS2
            for g in range(1, G):
                for j in range(4 * g, min(4 * (g + 1), T)):
                    emit_xr(j)
                for k, p in enumerate(_row_pairs(g)):
                    t1_pair(g, p)
                    exp_done[g].add(p)
                    slots_left = (G - g) * NP8 - k
                    remaining = total_mm - sum(t.emitted for t in tiles)
                    pump(-(-remaining // max(slots_left, 1)) + 1)
            pump(None)  # drain the tail
            wp = psum.tile([P, P], F32, tag="sm", bufs=4, name="tailwarm",
                           padded_shape=(P, 512))
            nc.tensor.matmul(
                wp[:], lhsT=ident8[:], rhs=ident8[:], start=True, stop=True
            )

    nc.compile()
    return nc


_NC_CACHE = None
_RUNNER = None


def _make_runner(nc):
    """Build the sharded PJRT callable once (mirrors bass2jax's
    run_bass_via_pjrt) so repeat calls skip jit retracing."""
    import jax
    from jax.sharding import Mesh, PartitionSpec

    from jax.experimental.shard_map import shard_map

    import concourse.bass2jax as b2j
    from concourse import mybir as _mybir

    b2j.install_neuronx_cc_hook()
    partition_name = (
        nc.partition_id_tensor.name if nc.partition_id_tensor else None
    )
    in_names, out_names, out_avals, zero_shapes = [], [], [], []
    for alloc in nc.m.functions[0].allocations:
        if not isinstance(alloc, _mybir.MemoryLocationSet):
            continue
        name = alloc.memorylocations[0].name
        if alloc.kind == "ExternalInput":
            if name != partition_name:
                in_names.append(name)
        elif alloc.kind == "ExternalOutput":
            out_names.append(name)
            shape = tuple(alloc.tensor_shape)
            dtype = _mybir.dt.np(alloc.dtype)
            out_avals.append(jax.core.ShapedArray(shape, dtype))
            zero_shapes.append(((B * shape[0],) + shape[1:], dtype))
    assert in_names == ["x"] and out_names == ["out"]
    n_params = len(in_names)
    all_in_names = list(in_names) + list(out_names)
    if partition_name is not None:
        all_in_names.append(partition_name)
    donate = tuple(range(n_params, n_params + len(out_names)))

    def _body(*args):
        operands = list(args)
        if partition_name is not None:
            operands.append(b2j.partition_id_tensor())
        outs = b2j._bass_exec_p.bind(
            *operands,
            out_avals=tuple(out_avals),
            in_names=tuple(all_in_names),
            out_names=tuple(out_names),
            lowering_input_output_aliases=(),
            sim_require_finite=True,
            sim_require_nnan=True,
            nc=nc,
        )
        return tuple(outs)

    devices = jax.devices()[:B]
    assert len(devices) == B
    mesh = Mesh(np.asarray(devices), ("core",))
    specs = (PartitionSpec("core"),)
    sharded = jax.jit(
        shard_map(
            _body,
            mesh=mesh,
            in_specs=specs * (n_params + len(out_names)),
            out_specs=specs * len(out_names),
            check_rep=False,
        ),
        donate_argnums=donate,
        keep_unused=True,
    )

    def run(x_full: np.ndarray) -> np.ndarray:
        zs = [np.zeros(s, d) for s, d in zero_shapes]
        out = sharded(np.ascontiguousarray(x_full.reshape(B * N, D)), *zs)
        return np.asarray(out[0]).reshape(B, N, D)

    return run


def kernel(inputs: np.ndarray) -> np.ndarray:
    global _NC_CACHE, _RUNNER
    if _NC_CACHE is None:
        _NC_CACHE = _build_nc()
    nc = _NC_CACHE
    inputs = np.ascontiguousarray(np.asarray(inputs, dtype=np.float32))
    assert inputs.shape == (B, N, D)
    if _RUNNER is None:
        try:
            _RUNNER = _make_runner(nc)
        except Exception:
            _RUNNER = False
    if _RUNNER:
        try:
            return _RUNNER(inputs)
        except Exception:
            pass
    in_maps = [{"x": inputs[i]} for i in range(B)]
    res = run_bass_kernel_spmd(nc, in_maps, list(range(B)))
    return np.stack([res.results[i]["out"] for i in range(B)], axis=0)


# revision 29
# speedup vs baseline: 1.0093x; 1.0093x over previous
"""Self-attention (Q=K=V) Trainium2 Bass kernel.

Full input: inputs [8, 2048, 256] fp32.  Output: softmax(X X^T / 16) X,
batched over dim 0.  Sharding: pure data-parallel - one batch element
per NeuronCore (8 cores), no collectives.

Per-core algorithm (X = [2048, 256]):
  - Load X into SBUF row-block tiles (plus two ones columns); X^T built
    on-chip with fp8 PE transposes (identity streams at 1 col/cycle).
  - Scores run in fp8e4 DoubleRow (one matmul contracts K=256); per
    column group g and row PAIR (2j, 2j+1) the two 512-wide matmuls land
    in adjacent PSUM banks and ONE 1024-wide exp drains them (the ~180
    cycle ACT instruction overhead is paid per 1024 elements, not 512).
    exp is biased by -ln(1024) so off-diagonal E/1024 fits fp8e4; the
    diagonal-group rows go to bf16 (e32) instead.
  - S is symmetric, so E's row-blocks double as the TRANSPOSED
    probability blocks stage 2 needs as stationary operands.
  - Stage 2 (per 128-query tile): 6 fp8 DoubleRow pair-matmuls
    (off-diagonal row pairs, ~4e-4 of softmax mass) + 4 bf16 matmuls
    (diagonal-group rows, ~99.96% of the mass).  The ones columns
    accumulate the denominator in the same PSUM tile, bit-consistent
    with the numerator weights.  Scale by its reciprocal and DMA out.
  - Emission is readiness-driven: each phase's stage-2 work starts as
    soon as the needed exp rows exist (diag row-pairs go first in each
    phase), so the post-loop tail is only a few matmuls.  Dummy ident
    matmuls warm the PE clock gate and a dummy exp preloads the ACT
    table set during the input-DMA wait.
"""

import numpy as np

import concourse.bacc as bacc
import concourse.tile as tile
from concourse import mybir
from concourse.bass_utils import run_bass_kernel_spmd
from concourse.masks import make_identity

B = 8
N = 2048
D = 256
P = 128
T = N // P   # 16 row/column tiles
C = D // P   # 2 contraction chunks for the scores matmul
G = 4        # 512-wide column groups
GW = N // G  # 512
NP8 = 8      # row pairs
SCALE = 1.0 / 16.0  # 1/sqrt(D)
EBIAS = -6.931471805599453  # -ln(1024): off-diag E fits fp8e4 up to s~12.4

F32 = mybir.dt.float32
BF16 = mybir.dt.bfloat16
FP8 = mybir.dt.float8e4
DR = mybir.MatmulPerfMode.DoubleRow
EXPF = mybir.ActivationFunctionType.Exp


def _row_pairs(g):
    """Phase-g row-pair order: the two diagonal-group pairs first, so
    e32 rows for this group's output tiles are exp'd early."""
    dp = [2 * g, 2 * g + 1]
    return dp + [p for p in range(NP8) if p not in dp]


def _build_nc():
    nc = bacc.Bacc("TRN2", target_bir_lowering=False, debug=False, num_devices=B)
    x = nc.dram_tensor("x", [N, D], F32, kind="ExternalInput").ap()
    out = nc.dram_tensor("out", [N, D], F32, kind="ExternalOutput").ap()

    with tile.TileContext(nc) as tc:
        with (
            tc.tile_pool(name="big", bufs=1) as big,
            tc.tile_pool(name="small", bufs=1) as small,
            tc.tile_pool(name="psum", bufs=2, space="PSUM") as psum,
            tc.tile_pool(name="ot", bufs=8) as ot,
        ):
            # x_tiles[j][p, 0:256] = X[j*128+p, :]; cols 256:258 = 1.0
            x_tiles = [
                big.tile([P, D + 2], F32, name=f"xj{j}", tag=f"x{j}")
                for j in range(T)
            ]
            xr_tiles = [
                big.tile([P, D + 2], BF16, name=f"xr{j}", tag=f"xr{j}")
                for j in range(T)
            ]
            # X^T: xt[p, c, n] = X[n, c*128+p]
            xt_sb = big.tile([P, C, N], FP8, tag="xt")
            # E/1024 storage: the diagonal-group tile of each row is bf16
            # (carries ~99.96% of softmax mass); off-diagonal tiles fp8.
            e32 = big.tile([P, T, GW], BF16, tag="e32")
            e8 = big.tile([P, T, N], FP8, tag="e8")
            # fp8 x pairs for DoubleRow stage-2: x8p[v][p,h,:] = row 2v+h
            x8p = [
                big.tile([P, 2, D + 2], FP8, name=f"x8p{v}", tag=f"x8p{v}")
                for v in range(NP8)
            ]

            ident32 = small.tile([P, P], F32, tag="id32")
            ident8 = small.tile([P, P], FP8, tag="id8")
            ebias = small.tile([P, 1], F32, tag="eb")
            wsrc = small.tile([P, P], FP8, tag="wsrc")

            # gpsimd's Q7 cores take ~6us to boot, so only the identity
            # (first needed by the ~6.3us transposes) may depend on it;
            # warmup-matmul and exp constants come from the DVE.
            nc.vector.memset(wsrc[:], 1.0)
            nc.vector.memset(ebias[:], EBIAS)
            make_identity(nc, ident32)
            nc.vector.tensor_copy(ident8[:], ident32[:])

            # Input DMAs.  Only sync/scalar/gpsimd queues can issue DMAs
            # and one dma_start transfers on ONE of the 16 DMA engines
            # (~22.5 GB/s, so a full 131KB tile takes ~5.8us).  Early
            # tiles are split into partition halves on two queues; the
            # scalar queue must stay clear for the exp stream, so its few
            # issues are interleaved between the first exps (below).
            xv = x.rearrange("(t p) d -> p t d", p=P)

            def in_dma(eng, j, p0, p1):
                eng.dma_start(out=x_tiles[j][p0:p1, 0:D], in_=xv[p0:p1, j, :])

            H = 64
            # both halves of each tile issued back-to-back so two DMA
            # engines transfer a tile in parallel (~2.75us instead of ~5.8)
            for j in [0, 2, 4, 6, 8, 10, 12, 14]:
                in_dma(nc.sync, j, 0, H)
                in_dma(nc.sync, j, H, P)
            for j in [1, 3, 5]:
                in_dma(nc.scalar, j, 0, H)
                in_dma(nc.scalar, j, H, P)
            # gpsimd boots ~6us in; its issues are slower (SWDGE ~1us)
            for j in [7, 9, 15, 11, 13]:
                in_dma(nc.gpsimd, j, 0, H)
                in_dma(nc.gpsimd, j, H, P)

            for j in range(T):
                nc.vector.memset(x_tiles[j][:, D : D + 2], 1.0)

            # PE clock-gate warmup: dummy matmuls keep the PE busy through
            # the HAM activity window during the DMA wait, so the real
            # matmul stream starts at 2.4 GHz.
            warm_n = [0]

            def warm(n):
                # dummy matmuls keep the PE inside the HAM activity window
                # (idle >3.4us re-throttles the clock to 1.2 GHz)
                for _ in range(n):
                    w = warm_n[0]
                    warm_n[0] += 1
                    wp = psum.tile([P, P], F32, tag="sm", bufs=4,
                                   name=f"wp{w}", padded_shape=(P, 512))
                    nc.tensor.matmul(
                        wp[:], lhsT=wsrc[:], rhs=wsrc[:], start=True,
                        stop=True,
                    )

            warm(40)

            xr_emitted = set()

            def emit_xr(j):
                # xr casts are lazy (phase-ahead of their stage-2 use) to
                # keep them out of the phase-0 DVE stream.  They must NOT
                # go on gpsimd: concurrent gpsimd+DVE reads of the same
                # x_tiles quarter the DVE cast rate (SBUF port contention).
                if j not in xr_emitted:
                    xr_emitted.add(j)
                    nc.vector.tensor_copy(xr_tiles[j][:], x_tiles[j][:])

            def load_step(j):
                # x8p (fp8) feeds both the stage-2 rhs and the fp8 PE
                # transposes that build X^T; xr (bf16) feeds stage-2 diag.
                nc.vector.tensor_copy(x8p[j // 2][:, j % 2, :], x_tiles[j][:])
                for c in range(C):
                    # fp8 transpose-mode requires an output element step of
                    # 2 in PSUM, hence the [P, P, 2] scratch written at
                    # stride 2.
                    pt = psum.tile([P, P, 2], FP8, tag="sm", bufs=4,
                                   name=f"pt{j}_{c}", padded_shape=(P, 1024, 2))
                    nc.tensor.transpose(
                        pt[:, :, 0], x8p[j // 2][:, j % 2, c * P : (c + 1) * P],
                        ident8[:],
                    )
                    nc.vector.tensor_copy(
                        xt_sb[:, c, j * P : (j + 1) * P], pt[:, :, 0]
                    )

            # Scores + exp for row pair (2p, 2p+1), column group g: two
            # DoubleRow matmuls into adjacent PSUM banks, one 1024-wide
            # exp over both.  MM and exp emission are split so each
            # pair's matmuls enter the PE queue one slot ahead of its exp
            # (stage-2 doses between them never gate the exp stream).
            ps_map = {}

            def t1_mms(g, p):
                r0 = 2 * p
                ps = psum.tile([P, 2, GW], F32, tag="pair", name=f"ps{g}_{p}")
                for h in range(2):
                    nc.tensor.matmul(
                        ps[:, h, :],
                        lhsT=xt_sb[:, :, (r0 + h) * P : (r0 + h + 1) * P],
                        rhs=xt_sb[:, :, g * GW : (g + 1) * GW],
                        start=True,
                        stop=True,
                        perf_mode=DR,
                    )
                ps_map[(g, p)] = ps

            def t1_exp(g, p):
                r0 = 2 * p
                ps = ps_map.pop((g, p))
                dst = (
                    e32[:, r0 : r0 + 2, :]
                    if p in (2 * g, 2 * g + 1)
                    else e8[:, r0 : r0 + 2, g * GW : (g + 1) * GW]
                )
                nc.scalar.activation(
                    out=dst, in_=ps[:, :, :], func=EXPF,
                    scale=SCALE, bias=ebias[:],
                )

            # ---- stage 2 bookkeeping -------------------------------------
            out_r = out.rearrange("(t p) d -> p t d", p=P)
            exp_done = [set() for _ in range(G)]  # pairs exp'd per phase
            NS2 = 10

            class S2:
                __slots__ = ("it", "g", "steps", "emitted", "po")

            tiles = []
            for it in range(T):
                g = it // 4
                t = S2()
                t.it, t.g, t.emitted, t.po = it, g, 0, None
                t.steps = [("bf", j) for j in range(4 * g, 4 * g + 4)] + [
                    ("f8", p) for p in _row_pairs(g) if p not in (2 * g, 2 * g + 1)
                ]
                tiles.append(t)

            out_engs = [nc.sync, nc.gpsimd]

            def close(t):
                it = t.it
                rl = ot.tile([P, 1], F32, tag="rl", name=f"rl{it}")
                nc.vector.reciprocal(rl[:], t.po[:, D : D + 1])
                o_t = ot.tile([P, D], F32, tag="ot", name=f"o{it}")
                if it < T - 4:
                    nc.vector.tensor_scalar_mul(o_t[:], t.po[:, 0:D], rl[:])
                    eng = out_engs[it % 2]
                    eng.dma_start(out=out_r[:, it, :], in_=o_t[:])
                else:
                    # the last 4 tiles close after the final exp: normalize
                    # on the now-idle scalar engine so the 4 closes don't
                    # serialize on the DVE, and spread the half-tile DMA
                    # issues over three queues so scalar's COPY stream and
                    # the issue serialization don't add up
                    nc.scalar.mul(o_t[:], t.po[:, 0:D], rl[:])
                    beng = nc.gpsimd if it < T - 2 else nc.scalar
                    for (q0, q1), eng in [((0, H), nc.sync), ((H, P), beng)]:
                        eng.dma_start(
                            out=out_r[q0:q1, it, :], in_=o_t[q0:q1, :]
                        )

            def step_ready(t, s):
                kind, v = t.steps[s]
                pair = v // 2 if kind == "bf" else v
                return pair in exp_done[t.g]

            def emit_one(t):
                kind, v = t.steps[t.emitted]
                first = t.emitted == 0
                last = t.emitted == NS2 - 1
                if first:
                    t.po = psum.tile([P, D + 2], F32, tag="sm", bufs=4,
                                     name=f"po{t.it}", padded_shape=(P, 512))
                if kind == "f8":
                    nc.tensor.matmul(
                        t.po[:],
                        lhsT=e8[:, 2 * v : 2 * v + 2,
                                t.it * P : (t.it + 1) * P],
                        rhs=x8p[v][:],
                        start=first, stop=last, perf_mode=DR,
                    )
                else:
                    lo = (t.it % 4) * P
                    nc.tensor.matmul(
                        t.po[:],
                        lhsT=e32[:, v, lo : lo + P],
                        rhs=xr_tiles[v][:],
                        start=first, stop=last,
                    )
                t.emitted += 1
                if t.emitted == NS2:
                    close(t)

            def open_count():
                return sum(1 for t in tiles if 0 < t.emitted < NS2)

            def pump(budget, max_open=4):
                emitted = 0
                while budget is None or emitted < budget:
                    progressed = False
                    for t in tiles:
                        if t.emitted >= NS2:
                            continue
                        if t.emitted == 0 and open_count() >= max_open:
                            break
                        if step_ready(t, t.emitted):
                            emit_one(t)
                            emitted += 1
                            progressed = True
                            break
                    if not progressed:
                        break
                return emitted

            # ---- schedule ------------------------------------------------
            # Flattened pair sequence with 1-slot-ahead MM emission.
            # Phase 0 (i<8) is input-limited: loads interleave, warmups
            # fill the PE until the load scratch frees up, then group-0
            # stage-2 starts eagerly in the remaining idle time.
            pair_seq = [(g, p) for g in range(G) for p in _row_pairs(g)]
            total_mm = T * NS2
            loads_done = [0]

            def loads_upto(n):
                while loads_done[0] < min(n, T):
                    load_step(loads_done[0])
                    loads_done[0] += 1

            loads_upto(6)
            t1_mms(*pair_seq[0])
            for i, (g, p) in enumerate(pair_seq):
                t1_exp(g, p)
                exp_done[g].add(p)
                if p == _row_pairs(g)[0]:
                    # rows whose diag-group stage-2 matmuls start in this
                    # phase; cast their bf16 copies now
                    for j in range(4 * g, 4 * g + 4):
                        emit_xr(j)
                if i + 1 < len(pair_seq):
                    loads_upto(2 * i + 8)
                    t1_mms(*pair_seq[i + 1])
                if i < 5:
                    warm(3)
                elif i < 8:
                    pump(4, max_open=2)
                else:
                    slots_left = len(pair_seq) - i
                    remaining = total_mm - sum(t.emitted for t in tiles)
                    pump(-(-remaining // max(slots_left, 1)) + 1)
            pump(None)  # drain the tail
            wp = psum.tile([P, P], F32, tag="sm", bufs=4, name="tailwarm",
                           padded_shape=(P, 512))
            nc.tensor.matmul(
                wp[:], lhsT=ident8[:], rhs=ident8[:], start=True, stop=True
            )

    nc.compile()
    return nc


_NC_CACHE = None
_RUNNER = None


def _make_runner(nc):
    """Build the sharded PJRT callable once (mirrors bass2jax's
    run_bass_via_pjrt) so repeat calls skip jit retracing."""
    import jax
    from jax.sharding import Mesh, PartitionSpec

    from jax.experimental.shard_map import shard_map

    import concourse.bass2jax as b2j
    from concourse import mybir as _mybir

    b2j.install_neuronx_cc_hook()
    partition_name = (
        nc.partition_id_tensor.name if nc.partition_id_tensor else None
    )
    in_names, out_names, out_avals, zero_shapes = [], [], [], []
    for alloc in nc.m.functions[0].allocations:
        if not isinstance(alloc, _mybir.MemoryLocationSet):
            continue
        name = alloc.memorylocations[0].name
        if alloc.kind == "ExternalInput":
            if name != partition_name:
                in_names.append(name)
        elif alloc.kind == "ExternalOutput":
            out_names.append(name)
            shape = tuple(alloc.tensor_shape)
            dtype = _mybir.dt.np(alloc.dtype)
            out_avals.append(jax.core.ShapedArray(shape, dtype))
            zero_shapes.append(((B * shape[0],) + shape[1:], dtype))
    assert in_names == ["x"] and out_names == ["out"]
    n_params = len(in_names)
    all_in_names = list(in_names) + list(out_names)
    if partition_name is not None:
        all_in_names.append(partition_name)
    donate = tuple(range(n_params, n_params + len(out_names)))

    def _body(*args):
        operands = list(args)
        if partition_name is not None:
            operands.append(b2j.partition_id_tensor())
        outs = b2j._bass_exec_p.bind(
            *operands,
            out_avals=tuple(out_avals),
            in_names=tuple(all_in_names),
            out_names=tuple(out_names),
            lowering_input_output_aliases=(),
            sim_require_finite=True,
            sim_require_nnan=True,
            nc=nc,
        )
        return tuple(outs)

    devices = jax.devices()[:B]
    assert len(devices) == B
    mesh = Mesh(np.asarray(devices), ("core",))
    specs = (PartitionSpec("core"),)
    sharded = jax.jit(
        shard_map(
            _body,
            mesh=mesh,
            in_specs=specs * (n_params + len(out_names)),
            out_specs=specs * len(out_names),
            check_rep=False,
        ),
        donate_argnums=donate,
        keep_unused=True,
    )

    def run(x_full: np.ndarray) -> np.ndarray:
        zs = [np.zeros(s, d) for s, d in zero_shapes]
        out = sharded(np.ascontiguousarray(x_full.reshape(B * N, D)), *zs)
        return np.asarray(out[0]).reshape(B, N, D)

    return run


def kernel(inputs: np.ndarray) -> np.ndarray:
    global _NC_CACHE, _RUNNER
    if _NC_CACHE is None:
        _NC_CACHE = _build_nc()
    nc = _NC_CACHE
    inputs = np.ascontiguousarray(np.asarray(inputs, dtype=np.float32))
    assert inputs.shape == (B, N, D)
    if _RUNNER is None:
        try:
            _RUNNER = _make_runner(nc)
        except Exception:
            _RUNNER = False
    if _RUNNER:
        try:
            return _RUNNER(inputs)
        except Exception:
            pass
    in_maps = [{"x": inputs[i]} for i in range(B)]
    res = run_bass_kernel_spmd(nc, in_maps, list(range(B)))
    return np.stack([res.results[i]["out"] for i in range(B)], axis=0)
